# revision 1
# baseline (speedup 1.0000x reference)
"""Trainium2 Bass kernel for a dense transformer encoder layer.

Model (see reference):
    kqv = x @ W_kqv ; split k,q,v ; multi-head attention (H=8, Hd=64)
    h   = gelu(attn_out @ W1 + b1) ; ffn = h @ W2 + b2
    out = LayerNorm(ffn)*gamma + beta + mean-pooled residual of x

Sharding: 8 cores, fully data-parallel, no collectives.  Core c handles
batch n = c//4 and query-row block qb = c%4 (512 rows).  K/V are computed
per-core over the full 2048 keys of the core's batch (duplicated within
each 4-core group, which avoids any inter-core communication).

Layout strategy ("transposed attention"): all attention tensors are kept
with the head-dim / feature-dim on partitions so that no on-chip
transposes are ever needed:
    qT,kT : [Hd, rows]   from  W.T @ x.T  (x.T staged by host)
    sT    : [keys, qrows] = kT_tile.T @ qT      (softmax along partitions
            handled by ones-column trick below; exp along free dim is not
            needed since sT has queries on the free dim)
    exp(sT) with no max-subtraction (scores are O(1); mask values only
            shift scores down for typical masks)
    outT  : v_aug.T @ exp(sT) accumulated over key tiles, where v_aug has
            a ones column => row 64 of the PSUM tile is the softmax
            denominator for each query.
    outT (= attention output transposed) is exactly the lhsT layout the
    FFN matmuls need, so the whole network runs transpose-free.

All matmuls run in bf16 (fp32 PSUM accumulation).  The host stages
pre-transposed / pre-cast operands; mask is staged transposed in bf16.
"""

import numpy as np
import ml_dtypes

import concourse.bass as bass
import concourse.mybir as mybir
import concourse.tile as tile
from concourse import bacc

F32 = mybir.dt.float32
BF16 = mybir.dt.bfloat16
AF = mybir.ActivationFunctionType
ALU = mybir.AluOpType

N, L, D, H, HD, DFF, DOUT = 2, 2048, 512, 8, 64, 2048, 256
NCORES = 8
LQ = N * L // NCORES          # 512 query rows per core
KT = L // 128                 # 16 key tiles
DCH = D // 128                # 4 contraction chunks of D
FBLK = DFF // 128             # 16 dff blocks
QTL = LQ // 128               # 4 query sub-tiles (output rows)
LN_EPS = 1e-5

# Replace Gelu with a sim-supported function when validating in CoreSim
# (CoreSim has no Gelu; hardware does).  Never enabled in production.
GELU_FUNC = AF.Gelu


def _emit(nc, reps=1):
    """Emit the whole per-core program under a TileContext."""
    dp = nc.declare_dram_parameter
    xt = dp("xt", [DCH, 128, L], BF16, isOutput=False)          # x[n].T, D-chunked
    xtq = dp("xtq", [DCH, 128, LQ], BF16, isOutput=False)       # x[n].T q-cols
    xq = dp("xq", [QTL, 128, D], F32, isOutput=False)           # x q-rows (residual)
    maskT = dp("maskT", [H, KT // 4, 128, 4, LQ], BF16, isOutput=False)  # mask^T, 4 key tiles per DMA group
    wkqv = dp("wkqv", [DCH, 128, 3 * D], BF16, isOutput=False)
    w1 = dp("w1", [DCH, 128, DFF], BF16, isOutput=False)
    w2 = dp("w2", [FBLK, 128, DOUT], BF16, isOutput=False)
    b1c = dp("b1c", [128, FBLK], F32, isOutput=False)
    b2r = dp("b2r", [128, DOUT], F32, isOutput=False)
    gamma = dp("gamma", [128, DOUT], F32, isOutput=False)
    beta = dp("beta", [128, DOUT], F32, isOutput=False)
    out = dp("out", [QTL, 128, DOUT], F32, isOutput=True)

    def bcast_ap(dram_1d, parts):
        ap = dram_1d[:]
        return bass.AP(tensor=ap.tensor, offset=ap.offset,
                       ap=[[0, parts]] + list(ap.ap))

    with tile.TileContext(nc) as tc:
      for _rep in range(reps):
        with (
            tc.tile_pool(name="const", bufs=1) as const,
            tc.tile_pool(name="mask", bufs=3) as maskp,
            tc.tile_pool(name="sexp", bufs=2) as sexpp,
            tc.tile_pool(name="norm", bufs=2) as normp,
            tc.tile_pool(name="ps_s", bufs=4, space="PSUM") as ps_s,
            tc.tile_pool(name="ps_o", bufs=2, space="PSUM") as ps_o,
        ):
            # ---------------- constant / input loads ----------------
            xt_sb = const.tile([128, DCH, L], BF16)
            xtq_sb = const.tile([128, DCH, LQ], BF16)
            xq_sb = const.tile([128, QTL, D], F32)
            wkqv_sb = const.tile([128, DCH, 3 * D], BF16)
            w1_sb = const.tile([128, DCH, DFF], BF16)
            w2_sb = const.tile([128, FBLK, DOUT], BF16)
            b1_sb = const.tile([128, FBLK], F32)
            b2b_sb = const.tile([128, DOUT], F32)
            gamma_sb = const.tile([128, DOUT], F32)
            beta_sb = const.tile([128, DOUT], F32)
            eps_sb = const.tile([128, 1], F32)
            ident_sb = const.tile([128, 128], BF16, name="ident")

            # chunked loads so compute can start on the first chunk
            for ch in range(DCH):
                nc.gpsimd.dma_start(xt_sb[:, ch, :], xt[ch])
                nc.gpsimd.dma_start(wkqv_sb[:, ch, :], wkqv[ch])
            nc.gpsimd.dma_start(xtq_sb, xtq.rearrange("c p l -> p c l"))
            for ch in range(DCH):
                nc.gpsimd.dma_start(w1_sb[:, ch, :], w1[ch])
            nc.gpsimd.dma_start(w2_sb, w2.rearrange("f p d -> p f d"))
            nc.gpsimd.dma_start(xq_sb, xq.rearrange("t p d -> p t d"))
            nc.gpsimd.dma_start(b1_sb, b1c[:])
            nc.gpsimd.dma_start(b2b_sb, b2r[:])
            nc.gpsimd.dma_start(gamma_sb, gamma[:])
            nc.gpsimd.dma_start(beta_sb, beta[:])
            nc.vector.memset(eps_sb, LN_EPS)
            from concourse.masks import make_identity
            make_identity(nc, ident_sb)

            kT_sb = const.tile([128, DCH, L], BF16, name="kT")
            qT_sb = const.tile([128, DCH, LQ], BF16, name="qT")
            attn_sb = const.tile([128, DCH, LQ], BF16, name="attn")
            v_sb = []

            def emit_v(ps_pool):
                for kt in range(KT):
                    ps = ps_pool.tile([128, D], F32, name="ps_qkv")
                    for ch in range(DCH):
                        nc.tensor.matmul(ps, xt_sb[:, ch, kt * 128:(kt + 1) * 128],
                                         wkqv_sb[:, ch, 2 * D:3 * D],
                                         start=(ch == 0), stop=(ch == DCH - 1))
                    vt = const.tile([128, H, HD + 1], BF16, name=f"v_{kt}")
                    nc.scalar.activation(vt[:, :, 0:HD],
                                         ps.rearrange("p (h d) -> p h d", h=H),
                                         AF.Copy)
                    nc.vector.memset(vt[:, :, HD:HD + 1], 1.0)
                    v_sb.append(vt)

            def emit_kT(ps_pool, ob):
                for lb in range(L // 512):
                    ps = ps_pool.tile([128, 512], F32, name="ps_qkv")
                    for ch in range(DCH):
                        nc.tensor.matmul(
                            ps, wkqv_sb[:, ch, ob * 128:(ob + 1) * 128],
                            xt_sb[:, ch, lb * 512:(lb + 1) * 512],
                            start=(ch == 0), stop=(ch == DCH - 1))
                    nc.vector.tensor_copy(kT_sb[:, ob, lb * 512:(lb + 1) * 512], ps)

            def emit_qT(ps_pool, ob):
                ps = ps_pool.tile([128, LQ], F32, name="ps_qkv")
                for ch in range(DCH):
                    nc.tensor.matmul(ps,
                                     wkqv_sb[:, ch, D + ob * 128:D + (ob + 1) * 128],
                                     xtq_sb[:, ch, :],
                                     start=(ch == 0), stop=(ch == DCH - 1))
                nc.scalar.activation(qT_sb[:, ob, :], ps, AF.Copy,
                                     scale=1.0 / np.sqrt(HD))

            def emit_norm(h, o_ps):
                ob, po = h // 2, (h % 2) * 64
                osb = normp.tile([128, LQ], F32, name="osb")
                nc.vector.tensor_copy(osb[0:HD + 1, :], o_ps[0:HD + 1, :])
                nc.vector.reciprocal(osb[HD:HD + 1, :], osb[HD:HD + 1, :])
                recipB = normp.tile([128, LQ], F32, name="recipB")
                rsrc = osb[HD:HD + 1, :]
                rap = list(rsrc.ap)
                nc.gpsimd.dma_start(
                    recipB[po:po + 64, :],
                    bass.AP(tensor=rsrc.tensor, offset=rsrc.offset,
                            ap=[list(rap[0]), [0, 64]] + [list(a) for a in rap[1:]]))
                if po == 0:
                    nc.vector.tensor_mul(attn_sb[0:64, ob, :],
                                         osb[0:64, :], recipB[0:64, :])
                else:
                    stage = normp.tile([128, LQ], F32, name="stage")
                    nc.gpsimd.dma_start(stage[64:128, :], osb[0:64, :])
                    nc.vector.tensor_mul(attn_sb[64:128, ob, :],
                                         stage[64:128, :], recipB[64:128, :])

            def emit_head(h):
                """Stage 1: stream scores->mask->exp for all 16 key tiles into
                SBUF (short 2-hop chains, deep buffering).  Stage 2: pure-PE
                burst of the 16 accumulating attn@v matmuls.  Heads pipeline:
                head h's stage-2 runs on PE while head h+1's stage-1 exp
                chains drain on DVE/ACT."""
                ob, po = h // 2, (h % 2) * 64
                e_tiles = []
                for g in range(KT // 4):
                    m_sb = maskp.tile([128, 4, LQ], BF16, name="m")
                    nc.sync.dma_start(m_sb, maskT[h, g])
                    for k in range(4):
                        kt = g * 4 + k
                        s_ps = ps_s.tile([128, LQ], F32, name="s_ps")
                        nc.tensor.matmul(s_ps,
                                         kT_sb[po:po + 64, ob,
                                               kt * 128:(kt + 1) * 128],
                                         qT_sb[po:po + 64, ob, :],
                                         start=True, stop=True)
                        nc.vector.tensor_add(s_ps, s_ps, m_sb[:, k, :])
                        e_sb = sexpp.tile([128, LQ], BF16, name=f"e_{kt}")
                        nc.scalar.activation(e_sb, s_ps, AF.Exp)
                        e_tiles.append(e_sb)
                o_ps = ps_o.tile([128, LQ], F32, name="o_ps")
                for kt in range(KT):
                    nc.tensor.matmul(o_ps[:HD + 1, :], v_sb[kt][:, h, :],
                                     e_tiles[kt], start=(kt == 0),
                                     stop=(kt == KT - 1))
                emit_norm(h, o_ps)

            # qkv psum pool scoped: closes before the FFN pools open so the
            # FFN psum banks only wait on (early) qkv reads, not attention
            with tc.tile_pool(name="ps_qkv", bufs=2, space="PSUM") as ps_qkv:
                emit_v(ps_qkv)
                for ob in range(DCH):
                    emit_kT(ps_qkv, ob)
                    emit_qT(ps_qkv, ob)

            # ---------------- attention + FFN (overlapping pools) ----------
            with (
                tc.tile_pool(name="hbuf", bufs=1) as hpool,
                tc.tile_pool(name="ffn", bufs=2) as ffnp,
                tc.tile_pool(name="ps_f", bufs=2, space="PSUM") as ps_f1,
            ):
                for h in range(H):
                    emit_head(h)

                h_sb = []
                for fb in range(FBLK):
                    ps = ps_f1.tile([128, LQ], F32, name="ps_h")
                    for ch in range(DCH):
                        nc.tensor.matmul(ps, w1_sb[:, ch, fb * 128:(fb + 1) * 128],
                                         attn_sb[:, ch, :],
                                         start=(ch == 0), stop=(ch == DCH - 1))
                    ht = hpool.tile([128, LQ], BF16, name=f"h_{fb}")
                    nc.scalar.activation(ht, ps, GELU_FUNC, bias=b1_sb[:, fb:fb + 1])
                    h_sb.append(ht)

                for qt in range(QTL):
                    ps2 = ps_f1.tile([128, DOUT], F32, name="ps_h")
                    for fb in range(FBLK):
                        nc.tensor.matmul(ps2, h_sb[fb][:, qt * 128:(qt + 1) * 128],
                                         w2_sb[:, fb, :],
                                         start=(fb == 0), stop=(fb == FBLK - 1))
                    nc.vector.tensor_add(ps2, ps2, b2b_sb)
                    stats = ffnp.tile([128, 6], F32, name="stats")
                    nc.vector.bn_stats(stats, ps2)
                    mv = ffnp.tile([128, 2], F32, name="mv")
                    nc.vector.bn_aggr(mv, stats)
                    sd = ffnp.tile([128, 1], F32, name="sd")
                    nc.scalar.activation(sd, mv[:, 1:2], AF.Sqrt, bias=eps_sb)
                    rstd = ffnp.tile([128, 1], F32, name="rstd")
                    nc.vector.reciprocal(rstd, sd)
                    t_sb = ffnp.tile([128, DOUT], F32, name="t")
                    nc.vector.tensor_scalar(t_sb, ps2, mv[:, 0:1], rstd,
                                            op0=ALU.subtract, op1=ALU.mult)
                    nc.vector.tensor_mul(t_sb, t_sb, gamma_sb)
                    r1 = ffnp.tile([128, DOUT], F32, name="r1")
                    nc.vector.tensor_add(r1, xq_sb[:, qt, 0:DOUT],
                                         xq_sb[:, qt, DOUT:D])
                    r2 = ffnp.tile([128, DOUT], F32, name="r2")
                    nc.vector.scalar_tensor_tensor(r2, r1, 0.5, beta_sb,
                                                   op0=ALU.mult, op1=ALU.add)
                    o_sb = ffnp.tile([128, DOUT], F32, name="o_sb")
                    nc.vector.tensor_add(o_sb, t_sb, r2)
                    nc.sync.dma_start(out[qt], o_sb)
    return nc


_NC = {}


def _get_nc(reps=1):
    if reps not in _NC:
        nc = bacc.Bacc()
        _emit(nc, reps)
        nc.compile()
        _NC[reps] = nc
    return _NC[reps]


def _stage_inputs(x, attn_mask, W_kqv, W1, b1, W2, b2, gamma, beta):
    """Build the 8 per-core input maps (host-side layout/dtype staging)."""
    bf = ml_dtypes.bfloat16
    x = np.asarray(x, np.float32)
    attn_mask = np.asarray(attn_mask, np.float32)
    shared = {
        "wkqv": np.ascontiguousarray(
            np.asarray(W_kqv, np.float32).reshape(DCH, 128, 3 * D)).astype(bf),
        "w1": np.ascontiguousarray(
            np.asarray(W1, np.float32).reshape(DCH, 128, DFF)).astype(bf),
        "w2": np.ascontiguousarray(
            np.asarray(W2, np.float32).reshape(FBLK, 128, DOUT)).astype(bf),
        "b1c": np.ascontiguousarray(
            np.asarray(b1, np.float32).reshape(FBLK, 128).T),
        "b2r": np.tile(np.asarray(b2, np.float32).reshape(1, DOUT), (128, 1)),
        "gamma": np.tile(np.asarray(gamma, np.float32).reshape(1, DOUT), (128, 1)),
        "beta": np.tile(np.asarray(beta, np.float32).reshape(1, DOUT), (128, 1)),
    }
    in_maps = []
    for c in range(NCORES):
        n, qb = divmod(c, NCORES // N)
        q0 = qb * LQ
        xTn = np.ascontiguousarray(x[n].T)                     # [D, L] f32
        mt = np.ascontiguousarray(
            attn_mask[n, :, q0:q0 + LQ, :].transpose(0, 2, 1))  # [H, L, LQ]
        mt = mt.reshape(H, KT // 4, 4, 128, LQ).transpose(0, 1, 3, 2, 4)
        m = dict(shared)
        m["xt"] = xTn.reshape(DCH, 128, L).astype(bf)
        m["xtq"] = np.ascontiguousarray(xTn[:, q0:q0 + LQ]).reshape(
            DCH, 128, LQ).astype(bf)
        m["xq"] = np.ascontiguousarray(x[n, q0:q0 + LQ, :]).reshape(QTL, 128, D)
        m["maskT"] = np.ascontiguousarray(mt).astype(bf)
        in_maps.append(m)
    return in_maps


def kernel(x, attn_mask, W_kqv, W1, b1, W2, b2, gamma, beta, num_heads,
           _return_results=False, **_ignored):
    assert int(num_heads) == H
    from concourse.bass_utils import run_bass_kernel_spmd

    nc = _get_nc()
    in_maps = _stage_inputs(x, attn_mask, W_kqv, W1, b1, W2, b2, gamma, beta)
    res = run_bass_kernel_spmd(nc, in_maps, core_ids=list(range(NCORES)))
    full = np.empty((N, L, DOUT), np.float32)
    for c in range(NCORES):
        n, qb = divmod(c, NCORES // N)
        q0 = qb * LQ
        full[n, q0:q0 + LQ, :] = res.results[c]["out"].reshape(LQ, DOUT)
    if _return_results:
        return full, res
    return full



# revision 2
# speedup vs baseline: 5.1771x; 5.1771x over previous
"""Trainium2 Bass kernel v2 for the dense transformer encoder layer.

Differences from the baseline (kernel.py):
  * Fast path when attn_mask is all-zero (the staged problem always is):
    no mask DMA (-17MB/core) and no DVE mask-adds (-94us DVE busy).
  * Scores are exp'd from PSUM in [128, 1024] chunks (two banks / two
    key tiles per Activation instruction) to amortize ACT init cost.
    (fp8 attention was measured and rejected: e4m3 can't span the score
    range, e5m2's 2 mantissa bits push max rel err past the 2e-2 gate.)
  * Host staging folds the 1/sqrt(Hd) scale into W_q, precomputes the
    mean-pooled residual, and rotates x.T per-core so the query block is
    always columns [0:LQ] (removes the separate xtq load).
  * kT chunk computation for the next head pair is interleaved into the
    current head's score/attn@v stream to fill PE stalls while ACT exps.

General (nonzero) masks fall back to the original baseline program.
"""

import numpy as np
import ml_dtypes

import concourse.bass as bass
import concourse.mybir as mybir
import concourse.tile as tile
from concourse import bacc

F32 = mybir.dt.float32
BF16 = mybir.dt.bfloat16
AF = mybir.ActivationFunctionType
ALU = mybir.AluOpType

N, L, D, H, HD, DFF, DOUT = 2, 2048, 512, 8, 64, 2048, 256
NCORES = 8
LQ = N * L // NCORES          # 512 query rows per core
KT = L // 128                 # 16 key tiles
KP = KT // 2                  # 8 key-tile pairs
DCH = D // 128                # 4 contraction chunks of D
FBLK = DFF // 128             # 16 dff blocks
QTL = LQ // 128               # 4 query sub-tiles (output rows)
LN_EPS = 1e-5

GELU_FUNC = AF.Gelu


def _emit_fast(nc, reps=1):
    """Zero-mask program."""
    dp = nc.declare_dram_parameter
    xt = dp("xt", [DCH, 128, L], BF16, isOutput=False)       # x[n].T rotated: q cols first
    wkqv = dp("wkqv", [DCH, 128, 3 * D], BF16, isOutput=False)
    w1 = dp("w1", [DCH, 128, DFF], BF16, isOutput=False)
    w2 = dp("w2", [FBLK, 128, DOUT], BF16, isOutput=False)
    # b1 | b2 | gamma | beta packed in one host-staged tensor
    consts = dp("consts", [128, FBLK + 3 * DOUT], F32, isOutput=False)
    b2bf = dp("b2bf", [1, DOUT], BF16, isOutput=False)
    resid = dp("resid", [QTL, 128, DOUT], F32, isOutput=False)
    out = dp("out", [QTL, 128, DOUT], F32, isOutput=True)

    with tile.TileContext(nc) as tc:
      for _rep in range(reps):
        with (
            tc.tile_pool(name="const", bufs=1) as const,
            tc.tile_pool(name="sexp", bufs=4) as sexpp,
            tc.tile_pool(name="norm", bufs=2) as normp,
            tc.tile_pool(name="ps_o", bufs=2, space="PSUM") as ps_o,
            tc.tile_pool(name="ps_k", bufs=2, space="PSUM") as ps_k,
        ):
            # ---------------- input loads (HWDGE via sync: 625ns issue,
            # no compute-engine time; priority order = need order) --------
            xt_sb = const.tile([128, DCH, L], BF16)
            wkqv_sb = const.tile([128, DCH, 3 * D], BF16)
            w1_sb = const.tile([128, DCH, DFF], BF16)
            w2_sb = const.tile([128, FBLK, DOUT], BF16)
            consts_sb = const.tile([128, FBLK + 3 * DOUT], F32)
            resid_sb = const.tile([128, QTL, DOUT], F32)
            rb_sb = const.tile([128, QTL, DOUT], F32)
            b2bf_sb = const.tile([128, DOUT], BF16)
            ones_sb = const.tile([128, 128], BF16)
            eps_sb = const.tile([128, 1], F32)
            b1_sb = consts_sb[:, 0:FBLK]
            b2b_sb = consts_sb[:, FBLK:FBLK + DOUT]
            gamma_sb = consts_sb[:, FBLK + DOUT:FBLK + 2 * DOUT]
            beta_sb = consts_sb[:, FBLK + 2 * DOUT:FBLK + 3 * DOUT]

            def load_xt(lb):
                nc.sync.dma_start(
                    xt_sb[:, :, lb * 512:(lb + 1) * 512],
                    xt[:, :, lb * 512:(lb + 1) * 512].rearrange("c p l -> p c l"))

            # W_k then x block 0 first: the kT chain starts ~3.5us in
            nc.sync.dma_start(wkqv_sb[:, :, 0:D],
                              wkqv[:, :, 0:D].rearrange("c p d -> p c d"))
            load_xt(0)
            nc.sync.dma_start(wkqv_sb[:, :, D:3 * D],
                              wkqv[:, :, D:3 * D].rearrange("c p d -> p c d"))
            for lb in range(1, L // 512):
                load_xt(lb)
            nc.sync.dma_start(w1_sb, w1.rearrange("c p f -> p c f"))
            nc.sync.dma_start(w2_sb, w2.rearrange("f p d -> p f d"))
            nc.sync.dma_start(consts_sb, consts[:])
            nc.sync.dma_start(resid_sb, resid.rearrange("t p d -> p t d"))
            nc.sync.dma_start(b2bf_sb[0:1, :], b2bf[:])
            nc.vector.memset(eps_sb, LN_EPS)
            nc.vector.memset(ones_sb[0:1, :], 1.0)
            # resid + beta precomputed off the LN tail path
            for qt in range(QTL):
                nc.vector.tensor_add(rb_sb[:, qt, :], resid_sb[:, qt, :],
                                     beta_sb)

            kT_sb = const.tile([128, DCH, L], BF16, name="kT")
            qT_sb = const.tile([128, DCH, LQ], BF16, name="qT")
            attn_sb = const.tile([128, DCH, LQ], BF16, name="attn")

            # v tiles: per head pair a 193-col block:
            #   [even data 0:64 | even ones @64 | odd pad | odd ones @97
            #    | odd data 129:193]
            # even lhsT = cols 0:65 -> out rows 0..64, denominator row 64.
            # odd lhsT = cols 65:193 (free 128, out base 0): denominator
            # lands on row 32 (a legal engine partition base) and data on
            # rows 64..127, partition-aligned with its attn_sb half; rows
            # 0..31 and 33..63 are garbage and never read.
            VW = 193
            v_sb = [const.tile([128, H // 2, VW], BF16, name=f"v_{kt}")
                    for kt in range(KT)]

            # ---------------- kqv ----------------
            def emit_v(kt):
                vt = v_sb[kt]
                ps = ps_k.tile([128, D], F32, name="ps_kqv")
                for ch in range(DCH):
                    nc.tensor.matmul(ps, xt_sb[:, ch, kt * 128:(kt + 1) * 128],
                                     wkqv_sb[:, ch, 2 * D:3 * D],
                                     start=(ch == 0), stop=(ch == DCH - 1))
                psh = ps.rearrange("p (h d) -> p h d", h=H)
                nc.vector.tensor_copy(vt[:, :, 0:HD], psh[:, 0:H:2, :])
                nc.vector.tensor_copy(vt[:, :, 129:193], psh[:, 1:H:2, :])
                nc.vector.memset(vt[:, :, HD:HD + 1], 1.0)
                nc.vector.memset(vt[:, :, 97:98], 1.0)

            def emit_kT_block(ob, lb):
                ps = ps_k.tile([128, D], F32, name="ps_kqv")
                for ch in range(DCH):
                    nc.tensor.matmul(
                        ps, wkqv_sb[:, ch, ob * 128:(ob + 1) * 128],
                        xt_sb[:, ch, lb * 512:(lb + 1) * 512],
                        start=(ch == 0), stop=(ch == DCH - 1))
                nc.vector.tensor_copy(kT_sb[:, ob, lb * 512:(lb + 1) * 512], ps)

            def emit_qT(ob):
                ps = ps_k.tile([128, LQ], F32, name="ps_kqv")
                for ch in range(DCH):
                    # W_q columns are pre-scaled by 1/sqrt(HD) on the host
                    nc.tensor.matmul(ps,
                                     wkqv_sb[:, ch, D + ob * 128:D + (ob + 1) * 128],
                                     xt_sb[:, ch, 0:LQ],
                                     start=(ch == 0), stop=(ch == DCH - 1))
                nc.vector.tensor_copy(qT_sb[:, ob, :], ps)

            def emit_norm(h, o_ps):
                """attn rows = o_ps rows * bcast(1/denominator row), all
                read straight from PSUM; the reciprocal row is broadcast
                across the 64 destination partitions by SBUF->SBUF DMA."""
                ob, po = h // 2, (h % 2) * 64
                dn = 32 if po else 64      # denominator partition in o_ps
                rec = normp.tile([128, LQ], F32, name="rec")
                nc.vector.reciprocal(rec[dn:dn + 1, :], o_ps[dn:dn + 1, :])
                recipB = normp.tile([128, LQ], F32, name="recipB")
                rsrc = rec[dn:dn + 1, :]
                rap = list(rsrc.ap)
                nc.sync.dma_start(
                    recipB[po:po + 64, :],
                    bass.AP(tensor=rsrc.tensor, offset=rsrc.offset,
                            ap=[list(rap[0]), [0, 64]] + [list(a) for a in rap[1:]]))
                nc.vector.tensor_mul(attn_sb[po:po + 64, ob, :],
                                     o_ps[po:po + 64, :], recipB[po:po + 64, :])

            def emit_scores(h, p, ps_s):
                ob, po = h // 2, (h % 2) * 64
                s_ps = ps_s.tile([128, 2 * LQ], F32, name="s_ps")
                for j in range(2):
                    kt = 2 * p + j
                    nc.tensor.matmul(s_ps[:, j * LQ:(j + 1) * LQ],
                                     kT_sb[po:po + 64, ob,
                                           kt * 128:(kt + 1) * 128],
                                     qT_sb[po:po + 64, ob, :],
                                     start=True, stop=True)
                e_sb = sexpp.tile([128, 2, LQ], BF16, name="e_sb")
                nc.scalar.activation(
                    e_sb.rearrange("p a b -> p (a b)"), s_ps, AF.Exp)
                return e_sb

            def emit_av(h, p, e_sb, o_ps):
                po = (h % 2) * 64
                orows = o_ps[0:128, :] if po else o_ps[0:65, :]
                vcols = (slice(65, 193) if po else slice(0, HD + 1))
                for j in range(2):
                    kt = 2 * p + j
                    nc.tensor.matmul(orows, v_sb[kt][:, h // 2, vcols],
                                     e_sb[:, j, :], start=(kt == 0),
                                     stop=(kt == KT - 1))

            def emit_head(h, ps_s, pe_filler=None):
                o_ps = ps_o.tile([128, LQ], F32, name="o_ps")
                for p in range(KP):
                    e_sb = emit_scores(h, p, ps_s)
                    if pe_filler:
                        pe_filler.pop(0)()
                    emit_av(h, p, e_sb, o_ps)
                emit_norm(h, o_ps)

            with tc.tile_pool(name="ps_s", bufs=2, space="PSUM") as ps_s:
                # ---- head 0 streams with kqv production; kT/qT first
                # (their weights arrive before W_v), score pairs next, the
                # v chains land between scores and their attn@v consumers.
                o_ps0 = ps_o.tile([128, LQ], F32, name="o_ps")
                for lb in range(L // 512):
                    emit_kT_block(0, lb)
                    if lb == 0:
                        emit_qT(0)
                    e0 = emit_scores(0, 2 * lb, ps_s)
                    e1 = emit_scores(0, 2 * lb + 1, ps_s)
                    for kt in range(4 * lb, 4 * lb + 4):
                        emit_v(kt)
                    emit_av(0, 2 * lb, e0, o_ps0)
                    emit_av(0, 2 * lb + 1, e1, o_ps0)
                emit_norm(0, o_ps0)

                # kT/qT for ob(c+1) spreads over heads 2c..2c+1 (it is
                # first consumed by head 2c+2), absorbing PE stall while
                # ACT works through the exps.
                def kq_fillers(ob):
                    f = [lambda lb=lb, ob=ob: emit_kT_block(ob, lb)
                         for lb in range(L // 512)]
                    f.append(lambda ob=ob: emit_qT(ob))
                    return f

                emit_head(1, ps_s, pe_filler=kq_fillers(1))
                f2 = kq_fillers(2)
                emit_head(2, ps_s, pe_filler=f2[:2])
                emit_head(3, ps_s, pe_filler=f2[2:])
                f3 = kq_fillers(3)
                emit_head(4, ps_s, pe_filler=f3[:2])
                emit_head(5, ps_s, pe_filler=f3[2:])
                emit_head(6, ps_s)
                emit_head(7, ps_s)

            # ---------------- FFN + LayerNorm ----------------
            # FFN1 rotates over 4 PSUM banks (ps_k + ps_o) so the gelu
            # read latency never stalls PE; FFN2/LN follow per qt.
            with (
                tc.tile_pool(name="hbuf", bufs=1) as hpool,
                tc.tile_pool(name="ffn", bufs=2) as ffnp,
            ):
                h_sb = []
                for fb in range(FBLK):
                    pool = ps_k if fb % 2 == 0 else ps_o
                    tag = "ps_kqv" if fb % 2 == 0 else "o_ps"
                    ps = pool.tile([128, LQ], F32, name=tag)
                    for ch in range(DCH):
                        nc.tensor.matmul(ps, w1_sb[:, ch, fb * 128:(fb + 1) * 128],
                                         attn_sb[:, ch, :],
                                         start=(ch == 0), stop=(ch == DCH - 1))
                    ht = hpool.tile([128, LQ], BF16, name=f"h_{fb}")
                    nc.scalar.activation(ht, ps, GELU_FUNC, bias=b1_sb[:, fb:fb + 1])
                    h_sb.append(ht)

                for qt in range(QTL):
                    pool = ps_k if qt % 2 == 0 else ps_o
                    tag = "ps_kqv" if qt % 2 == 0 else "o_ps"
                    ps2 = pool.tile([128, DOUT], F32, name=tag)
                    for fb in range(FBLK):
                        nc.tensor.matmul(ps2, h_sb[fb][:, qt * 128:(qt + 1) * 128],
                                         w2_sb[:, fb, :],
                                         start=(fb == 0), stop=False)
                    # + b2 broadcast: ones-row.T @ b2-row accumulates b2
                    nc.tensor.matmul(ps2, ones_sb[0:1, :], b2bf_sb[0:1, :],
                                     start=False, stop=True)
                    stats = ffnp.tile([128, 6], F32, name="stats")
                    nc.vector.bn_stats(stats, ps2)
                    mv = ffnp.tile([128, 2], F32, name="mv")
                    nc.vector.bn_aggr(mv, stats)
                    sd = ffnp.tile([128, 1], F32, name="sd")
                    nc.scalar.activation(sd, mv[:, 1:2], AF.Sqrt, bias=eps_sb)
                    rstd = ffnp.tile([128, 1], F32, name="rstd")
                    nc.vector.reciprocal(rstd, sd)
                    t_sb = ffnp.tile([128, DOUT], F32, name="t")
                    nc.vector.tensor_scalar(t_sb, ps2, mv[:, 0:1], rstd,
                                            op0=ALU.subtract, op1=ALU.mult)
                    t2 = ffnp.tile([128, DOUT], F32, name="t2")
                    nc.vector.tensor_mul(t2, t_sb, gamma_sb)
                    o_sb = ffnp.tile([128, DOUT], F32, name="o_sb")
                    nc.vector.tensor_add(o_sb, t2, rb_sb[:, qt, :])
                    nc.sync.dma_start(out[qt], o_sb)
    return nc


def _emit_masked(nc, reps=1):
    """Emit the whole per-core program under a TileContext."""
    dp = nc.declare_dram_parameter
    xt = dp("xt", [DCH, 128, L], BF16, isOutput=False)          # x[n].T, D-chunked
    xtq = dp("xtq", [DCH, 128, LQ], BF16, isOutput=False)       # x[n].T q-cols
    xq = dp("xq", [QTL, 128, D], F32, isOutput=False)           # x q-rows (residual)
    maskT = dp("maskT", [H, KT // 4, 128, 4, LQ], BF16, isOutput=False)  # mask^T, 4 key tiles per DMA group
    wkqv = dp("wkqv", [DCH, 128, 3 * D], BF16, isOutput=False)
    w1 = dp("w1", [DCH, 128, DFF], BF16, isOutput=False)
    w2 = dp("w2", [FBLK, 128, DOUT], BF16, isOutput=False)
    b1c = dp("b1c", [128, FBLK], F32, isOutput=False)
    b2r = dp("b2r", [128, DOUT], F32, isOutput=False)
    gamma = dp("gamma", [128, DOUT], F32, isOutput=False)
    beta = dp("beta", [128, DOUT], F32, isOutput=False)
    out = dp("out", [QTL, 128, DOUT], F32, isOutput=True)

    def bcast_ap(dram_1d, parts):
        ap = dram_1d[:]
        return bass.AP(tensor=ap.tensor, offset=ap.offset,
                       ap=[[0, parts]] + list(ap.ap))

    with tile.TileContext(nc) as tc:
      for _rep in range(reps):
        with (
            tc.tile_pool(name="const", bufs=1) as const,
            tc.tile_pool(name="mask", bufs=3) as maskp,
            tc.tile_pool(name="sexp", bufs=2) as sexpp,
            tc.tile_pool(name="norm", bufs=2) as normp,
            tc.tile_pool(name="ps_s", bufs=4, space="PSUM") as ps_s,
            tc.tile_pool(name="ps_o", bufs=2, space="PSUM") as ps_o,
        ):
            # ---------------- constant / input loads ----------------
            xt_sb = const.tile([128, DCH, L], BF16)
            xtq_sb = const.tile([128, DCH, LQ], BF16)
            xq_sb = const.tile([128, QTL, D], F32)
            wkqv_sb = const.tile([128, DCH, 3 * D], BF16)
            w1_sb = const.tile([128, DCH, DFF], BF16)
            w2_sb = const.tile([128, FBLK, DOUT], BF16)
            b1_sb = const.tile([128, FBLK], F32)
            b2b_sb = const.tile([128, DOUT], F32)
            gamma_sb = const.tile([128, DOUT], F32)
            beta_sb = const.tile([128, DOUT], F32)
            eps_sb = const.tile([128, 1], F32)
            ident_sb = const.tile([128, 128], BF16, name="ident")

            # chunked loads so compute can start on the first chunk
            for ch in range(DCH):
                nc.gpsimd.dma_start(xt_sb[:, ch, :], xt[ch])
                nc.gpsimd.dma_start(wkqv_sb[:, ch, :], wkqv[ch])
            nc.gpsimd.dma_start(xtq_sb, xtq.rearrange("c p l -> p c l"))
            for ch in range(DCH):
                nc.gpsimd.dma_start(w1_sb[:, ch, :], w1[ch])
            nc.gpsimd.dma_start(w2_sb, w2.rearrange("f p d -> p f d"))
            nc.gpsimd.dma_start(xq_sb, xq.rearrange("t p d -> p t d"))
            nc.gpsimd.dma_start(b1_sb, b1c[:])
            nc.gpsimd.dma_start(b2b_sb, b2r[:])
            nc.gpsimd.dma_start(gamma_sb, gamma[:])
            nc.gpsimd.dma_start(beta_sb, beta[:])
            nc.vector.memset(eps_sb, LN_EPS)
            from concourse.masks import make_identity
            make_identity(nc, ident_sb)

            kT_sb = const.tile([128, DCH, L], BF16, name="kT")
            qT_sb = const.tile([128, DCH, LQ], BF16, name="qT")
            attn_sb = const.tile([128, DCH, LQ], BF16, name="attn")
            v_sb = []

            def emit_v(ps_pool):
                for kt in range(KT):
                    ps = ps_pool.tile([128, D], F32, name="ps_qkv")
                    for ch in range(DCH):
                        nc.tensor.matmul(ps, xt_sb[:, ch, kt * 128:(kt + 1) * 128],
                                         wkqv_sb[:, ch, 2 * D:3 * D],
                                         start=(ch == 0), stop=(ch == DCH - 1))
                    vt = const.tile([128, H, HD + 1], BF16, name=f"v_{kt}")
                    nc.scalar.activation(vt[:, :, 0:HD],
                                         ps.rearrange("p (h d) -> p h d", h=H),
                                         AF.Copy)
                    nc.vector.memset(vt[:, :, HD:HD + 1], 1.0)
                    v_sb.append(vt)

            def emit_kT(ps_pool, ob):
                for lb in range(L // 512):
                    ps = ps_pool.tile([128, 512], F32, name="ps_qkv")
                    for ch in range(DCH):
                        nc.tensor.matmul(
                            ps, wkqv_sb[:, ch, ob * 128:(ob + 1) * 128],
                            xt_sb[:, ch, lb * 512:(lb + 1) * 512],
                            start=(ch == 0), stop=(ch == DCH - 1))
                    nc.vector.tensor_copy(kT_sb[:, ob, lb * 512:(lb + 1) * 512], ps)

            def emit_qT(ps_pool, ob):
                ps = ps_pool.tile([128, LQ], F32, name="ps_qkv")
                for ch in range(DCH):
                    nc.tensor.matmul(ps,
                                     wkqv_sb[:, ch, D + ob * 128:D + (ob + 1) * 128],
                                     xtq_sb[:, ch, :],
                                     start=(ch == 0), stop=(ch == DCH - 1))
                nc.scalar.activation(qT_sb[:, ob, :], ps, AF.Copy,
                                     scale=1.0 / np.sqrt(HD))

            def emit_norm(h, o_ps):
                ob, po = h // 2, (h % 2) * 64
                osb = normp.tile([128, LQ], F32, name="osb")
                nc.vector.tensor_copy(osb[0:HD + 1, :], o_ps[0:HD + 1, :])
                nc.vector.reciprocal(osb[HD:HD + 1, :], osb[HD:HD + 1, :])
                recipB = normp.tile([128, LQ], F32, name="recipB")
                rsrc = osb[HD:HD + 1, :]
                rap = list(rsrc.ap)
                nc.gpsimd.dma_start(
                    recipB[po:po + 64, :],
                    bass.AP(tensor=rsrc.tensor, offset=rsrc.offset,
                            ap=[list(rap[0]), [0, 64]] + [list(a) for a in rap[1:]]))
                if po == 0:
                    nc.vector.tensor_mul(attn_sb[0:64, ob, :],
                                         osb[0:64, :], recipB[0:64, :])
                else:
                    stage = normp.tile([128, LQ], F32, name="stage")
                    nc.gpsimd.dma_start(stage[64:128, :], osb[0:64, :])
                    nc.vector.tensor_mul(attn_sb[64:128, ob, :],
                                         stage[64:128, :], recipB[64:128, :])

            def emit_head(h):
                """Stage 1: stream scores->mask->exp for all 16 key tiles into
                SBUF (short 2-hop chains, deep buffering).  Stage 2: pure-PE
                burst of the 16 accumulating attn@v matmuls.  Heads pipeline:
                head h's stage-2 runs on PE while head h+1's stage-1 exp
                chains drain on DVE/ACT."""
                ob, po = h // 2, (h % 2) * 64
                e_tiles = []
                for g in range(KT // 4):
                    m_sb = maskp.tile([128, 4, LQ], BF16, name="m")
                    nc.sync.dma_start(m_sb, maskT[h, g])
                    for k in range(4):
                        kt = g * 4 + k
                        s_ps = ps_s.tile([128, LQ], F32, name="s_ps")
                        nc.tensor.matmul(s_ps,
                                         kT_sb[po:po + 64, ob,
                                               kt * 128:(kt + 1) * 128],
                                         qT_sb[po:po + 64, ob, :],
                                         start=True, stop=True)
                        nc.vector.tensor_add(s_ps, s_ps, m_sb[:, k, :])
                        e_sb = sexpp.tile([128, LQ], BF16, name=f"e_{kt}")
                        nc.scalar.activation(e_sb, s_ps, AF.Exp)
                        e_tiles.append(e_sb)
                o_ps = ps_o.tile([128, LQ], F32, name="o_ps")
                for kt in range(KT):
                    nc.tensor.matmul(o_ps[:HD + 1, :], v_sb[kt][:, h, :],
                                     e_tiles[kt], start=(kt == 0),
                                     stop=(kt == KT - 1))
                emit_norm(h, o_ps)

            # qkv psum pool scoped: closes before the FFN pools open so the
            # FFN psum banks only wait on (early) qkv reads, not attention
            with tc.tile_pool(name="ps_qkv", bufs=2, space="PSUM") as ps_qkv:
                emit_v(ps_qkv)
                for ob in range(DCH):
                    emit_kT(ps_qkv, ob)
                    emit_qT(ps_qkv, ob)

            # ---------------- attention + FFN (overlapping pools) ----------
            with (
                tc.tile_pool(name="hbuf", bufs=1) as hpool,
                tc.tile_pool(name="ffn", bufs=2) as ffnp,
                tc.tile_pool(name="ps_f", bufs=2, space="PSUM") as ps_f1,
            ):
                for h in range(H):
                    emit_head(h)

                h_sb = []
                for fb in range(FBLK):
                    ps = ps_f1.tile([128, LQ], F32, name="ps_h")
                    for ch in range(DCH):
                        nc.tensor.matmul(ps, w1_sb[:, ch, fb * 128:(fb + 1) * 128],
                                         attn_sb[:, ch, :],
                                         start=(ch == 0), stop=(ch == DCH - 1))
                    ht = hpool.tile([128, LQ], BF16, name=f"h_{fb}")
                    nc.scalar.activation(ht, ps, GELU_FUNC, bias=b1_sb[:, fb:fb + 1])
                    h_sb.append(ht)

                for qt in range(QTL):
                    ps2 = ps_f1.tile([128, DOUT], F32, name="ps_h")
                    for fb in range(FBLK):
                        nc.tensor.matmul(ps2, h_sb[fb][:, qt * 128:(qt + 1) * 128],
                                         w2_sb[:, fb, :],
                                         start=(fb == 0), stop=(fb == FBLK - 1))
                    nc.vector.tensor_add(ps2, ps2, b2b_sb)
                    stats = ffnp.tile([128, 6], F32, name="stats")
                    nc.vector.bn_stats(stats, ps2)
                    mv = ffnp.tile([128, 2], F32, name="mv")
                    nc.vector.bn_aggr(mv, stats)
                    sd = ffnp.tile([128, 1], F32, name="sd")
                    nc.scalar.activation(sd, mv[:, 1:2], AF.Sqrt, bias=eps_sb)
                    rstd = ffnp.tile([128, 1], F32, name="rstd")
                    nc.vector.reciprocal(rstd, sd)
                    t_sb = ffnp.tile([128, DOUT], F32, name="t")
                    nc.vector.tensor_scalar(t_sb, ps2, mv[:, 0:1], rstd,
                                            op0=ALU.subtract, op1=ALU.mult)
                    nc.vector.tensor_mul(t_sb, t_sb, gamma_sb)
                    r1 = ffnp.tile([128, DOUT], F32, name="r1")
                    nc.vector.tensor_add(r1, xq_sb[:, qt, 0:DOUT],
                                         xq_sb[:, qt, DOUT:D])
                    r2 = ffnp.tile([128, DOUT], F32, name="r2")
                    nc.vector.scalar_tensor_tensor(r2, r1, 0.5, beta_sb,
                                                   op0=ALU.mult, op1=ALU.add)
                    o_sb = ffnp.tile([128, DOUT], F32, name="o_sb")
                    nc.vector.tensor_add(o_sb, t_sb, r2)
                    nc.sync.dma_start(out[qt], o_sb)
    return nc



def _stage_masked(x, attn_mask, W_kqv, W1, b1, W2, b2, gamma, beta):
    """Build the 8 per-core input maps (host-side layout/dtype staging)."""
    bf = ml_dtypes.bfloat16
    x = np.asarray(x, np.float32)
    attn_mask = np.asarray(attn_mask, np.float32)
    shared = {
        "wkqv": np.ascontiguousarray(
            np.asarray(W_kqv, np.float32).reshape(DCH, 128, 3 * D)).astype(bf),
        "w1": np.ascontiguousarray(
            np.asarray(W1, np.float32).reshape(DCH, 128, DFF)).astype(bf),
        "w2": np.ascontiguousarray(
            np.asarray(W2, np.float32).reshape(FBLK, 128, DOUT)).astype(bf),
        "b1c": np.ascontiguousarray(
            np.asarray(b1, np.float32).reshape(FBLK, 128).T),
        "b2r": np.tile(np.asarray(b2, np.float32).reshape(1, DOUT), (128, 1)),
        "gamma": np.tile(np.asarray(gamma, np.float32).reshape(1, DOUT), (128, 1)),
        "beta": np.tile(np.asarray(beta, np.float32).reshape(1, DOUT), (128, 1)),
    }
    in_maps = []
    for c in range(NCORES):
        n, qb = divmod(c, NCORES // N)
        q0 = qb * LQ
        xTn = np.ascontiguousarray(x[n].T)                     # [D, L] f32
        mt = np.ascontiguousarray(
            attn_mask[n, :, q0:q0 + LQ, :].transpose(0, 2, 1))  # [H, L, LQ]
        mt = mt.reshape(H, KT // 4, 4, 128, LQ).transpose(0, 1, 3, 2, 4)
        m = dict(shared)
        m["xt"] = xTn.reshape(DCH, 128, L).astype(bf)
        m["xtq"] = np.ascontiguousarray(xTn[:, q0:q0 + LQ]).reshape(
            DCH, 128, LQ).astype(bf)
        m["xq"] = np.ascontiguousarray(x[n, q0:q0 + LQ, :]).reshape(QTL, 128, D)
        m["maskT"] = np.ascontiguousarray(mt).astype(bf)
        in_maps.append(m)
    return in_maps



_NC = {}


def _get_nc(reps=1, masked=False):
    key = (reps, masked)
    if key not in _NC:
        nc = bacc.Bacc()
        (_emit_masked if masked else _emit_fast)(nc, reps)
        nc.compile()
        _NC[key] = nc
    return _NC[key]


def _stage_inputs(x, W_kqv, W1, b1, W2, b2, gamma, beta):
    """Per-core input maps for the fast (zero-mask) program."""
    bf = ml_dtypes.bfloat16
    x = np.asarray(x, np.float32)
    Wk = np.asarray(W_kqv, np.float32).copy()
    Wk[:, D:2 * D] *= 1.0 / np.sqrt(HD)          # fold score scale into W_q
    consts = np.concatenate([
        np.asarray(b1, np.float32).reshape(FBLK, 128).T,
        np.tile(np.asarray(b2, np.float32).reshape(1, DOUT), (128, 1)),
        np.tile(np.asarray(gamma, np.float32).reshape(1, DOUT), (128, 1)),
        np.tile(np.asarray(beta, np.float32).reshape(1, DOUT), (128, 1)),
    ], axis=1)
    shared = {
        "wkqv": np.ascontiguousarray(Wk.reshape(DCH, 128, 3 * D)).astype(bf),
        "w1": np.ascontiguousarray(
            np.asarray(W1, np.float32).reshape(DCH, 128, DFF)).astype(bf),
        "w2": np.ascontiguousarray(
            np.asarray(W2, np.float32).reshape(FBLK, 128, DOUT)).astype(bf),
        "consts": np.ascontiguousarray(consts),
        "b2bf": np.asarray(b2, np.float32).reshape(1, DOUT).astype(bf),
    }
    in_maps = []
    for c in range(NCORES):
        n, qb = divmod(c, NCORES // N)
        q0 = qb * LQ
        # rotate keys so this core's query block is columns [0:LQ]; key
        # order is irrelevant to attention (sum over keys)
        xTn = np.roll(np.asarray(x[n].T), -q0, axis=1)         # [D, L] f32
        xq = x[n, q0:q0 + LQ, :]                               # [LQ, D] f32
        m = dict(shared)
        m["xt"] = np.ascontiguousarray(xTn).reshape(DCH, 128, L).astype(bf)
        m["resid"] = np.ascontiguousarray(
            0.5 * (xq[:, 0:DOUT] + xq[:, DOUT:D])).reshape(QTL, 128, DOUT)
        in_maps.append(m)
    return in_maps


def kernel(x, attn_mask, W_kqv, W1, b1, W2, b2, gamma, beta, num_heads,
           _return_results=False, **_ignored):
    assert int(num_heads) == H
    from concourse.bass_utils import run_bass_kernel_spmd

    if np.any(np.asarray(attn_mask)):
        nc = _get_nc(masked=True)
        in_maps = _stage_masked(x, attn_mask, W_kqv, W1, b1, W2, b2,
                                gamma, beta)
        res = run_bass_kernel_spmd(nc, in_maps, core_ids=list(range(NCORES)))
        full = np.empty((N, L, DOUT), np.float32)
        for c in range(NCORES):
            n, qb = divmod(c, NCORES // N)
            q0 = qb * LQ
            full[n, q0:q0 + LQ, :] = res.results[c]["out"].reshape(LQ, DOUT)
        if _return_results:
            return full, res
        return full

    nc = _get_nc()
    in_maps = _stage_inputs(x, W_kqv, W1, b1, W2, b2, gamma, beta)
    res = run_bass_kernel_spmd(nc, in_maps, core_ids=list(range(NCORES)))
    full = np.empty((N, L, DOUT), np.float32)
    for c in range(NCORES):
        n, qb = divmod(c, NCORES // N)
        q0 = qb * LQ
        full[n, q0:q0 + LQ, :] = res.results[c]["out"].reshape(LQ, DOUT)
    if _return_results:
        return full, res
    return full
